# revision 46
# baseline (speedup 1.0000x reference)
"""CondConv (routing -> per-sample mixed 3x3 conv -> frozen BN -> ReLU -> residual)
on 8 Trainium2 NeuronCores, data-parallel over batch (4 samples/core).

Structure (per core):
  - conv contraction split by precision: 3 taps (center + side edges) run
    as bf16 matmuls (2 cin tiles each), 6 taps (corners + top/bottom
    edges) run as fp8-e4m3 DoubleRow matmuls (one 256-deep matmul per
    tap, 2 MACs/cell/cycle) -> 12 PE ops per output chunk instead of 18.
    All weights are 64x-scaled so the mixed fp8 kernels sit in e4m3's
    normal range; the 1/64 is folded into the BN scale. Sample 0
    additionally streams the fp8 x copy for its bf16 taps, so the PE
    stream head depends only on one early DMA. Measured rel-err vs the
    fp32 reference: 1.834e-2 (run-deterministic; gate 2e-2).
  - preamble DMAs issue in need-time order with no gating: the sync
    queue's semaphore-lane recycling keeps ~8 transfers in flight, and
    the Tile framework coalesces the PE stream's per-lane waits to the
    max threshold of the emission window -- so the only thing that works
    is landing every tensor before its consumer (gating experiments all
    moved the stall instead of removing it). Wide tensors go as
    partition-split descriptor pairs (one descriptor tops out well below
    the HBM rate; per-partition packet size sets throughput).
  - routing reads the fp8 x copy (quantization shifts the logits ~1e-4):
    per cin tile ACT reduces columns 0:24 (Copy+accum) while DVE reduces
    24:58 with a pairwise-add tree; the routing dot runs as two parallel
    halves (ACT scaled-copy + DVE stt) in bf16, summed across partitions
    by two accumulating N=4 PE matmuls (single-pass bf16), sigmoid on ACT
    reads the PSUM directly.
  - per-sample mixed kernels on DVE (expert-pair trees); fp8 taps mixed
    as one [128, 6tap*2ct*128] slab with a final e4m3-rounded add.
  - conv chunks processed in 4/3 groups ([4x bf16][4x fp8+evac]...) --
    the first DoubleRow matmul after a bf16 run pays ~170ns, so grouping
    cuts the weight-path switches; group sizes keep current + still-
    evacuating PSUM banks within the pool. The warm-up tile shares the
    acc tag, so all 8 PSUM banks serve the steady-state rotation. The
    final sweep runs its tiny bottom chunks first and ends on a 4-row
    chunk so the tail is one short ACT+DVE+DMA chain.
  - warm-up dummy matmuls keep the PE HAM clock window busy from t=0,
    gated on the x8 DMA so the PE tracks preamble progress under jitter.
  - BN(frozen)+ReLU fused into the ACT PSUM evacuation (Newton rsqrt on
    DVE), residual add on DVE, bf16 output upcast to fp32 on host.
"""

import threading

import ml_dtypes
import numpy as np

import concourse.bass as bass
import concourse.mybir as mybir
import concourse.tile as tile
from concourse import bacc, bass_isa
from concourse.bass_utils import run_bass_kernel_spmd

F32 = mybir.dt.float32
BF16 = mybir.dt.bfloat16
E4 = mybir.dt.float8e4
AX = mybir.AxisListType
OP = mybir.AluOpType
AF = mybir.ActivationFunctionType
DR = mybir.MatmulPerfMode.DoubleRow

N_CORES = 8
B, CIN, COUT, H, W, KS, E = 32, 256, 256, 56, 56, 3, 4
BPC = B // N_CORES  # samples per core
CT = CIN // 128     # cin partition tiles
OTN = COUT // 128   # cout partition tiles
WP = W + 2          # width zero-padded (kj shifts); height handled by clipping
XO = 1              # column where the real image starts
RC = 7              # row chunks per image
RH = H // RC        # rows per chunk
NF = RH * W         # moving-dim elements per matmul
BN_EPS = 1e-5
WSCALE = 64.0       # weight pre-scale so e4m3 mixed kernels avoid subnormals

# taps: 3 bf16 (center first -- it covers the full chunk, so it carries
# start=True), 6 fp8 (DoubleRow: both cin tiles in one matmul)
TAPS16 = [(1, 1), (1, 0), (1, 2)]
TAPS8 = [(0, 0), (0, 2), (2, 0), (2, 2), (2, 1), (0, 1)]
NT16 = len(TAPS16)
NT8 = len(TAPS8)

# warm-up dummy matmul counts (N=448 each; block A ungated, B gated on
# the x8 DMA) and N=64 fillers bridging the sigmoid+mix window
WARM_A, WARM_B, FILLERS = 10, 8, 28


def build_bass():
    nc = bacc.Bacc("TRN2", target_bir_lowering=False, debug=False)

    x_d = nc.dram_tensor("x", [BPC, CIN, H, WP], BF16, kind="ExternalInput")
    x8_d = nc.dram_tensor("x8", [BPC, 128, CT, H, WP], E4,
                          kind="ExternalInput")
    wt_d = nc.dram_tensor("wt", [CT, 128, OTN, NT16, E, 128], BF16,
                          kind="ExternalInput")
    w8_d = nc.dram_tensor("w8", [128, OTN, NT8, E, CT, 128], BF16,
                          kind="ExternalInput")
    pp_d = nc.dram_tensor("pp", [128, 20], F32, kind="ExternalInput")
    y_d = nc.dram_tensor("y", [BPC, COUT, H, W], BF16, kind="ExternalOutput")

    x_ap = x_d.ap()
    x8_ap = x8_d.ap()
    wt_ap = wt_d.ap()
    w8_ap = w8_d.ap()
    pp_ap = pp_d.ap()
    y_ap = y_d.ap()

    with tile.TileContext(nc) as tc:
        with (
            tc.tile_pool(name="wbp", bufs=1) as wbp,
            tc.tile_pool(name="xpp", bufs=1) as xpp,
            tc.tile_pool(name="mwp", bufs=1) as mwp,
            tc.tile_pool(name="otp", bufs=10) as otp,
            tc.tile_pool(name="snp", bufs=1) as snp,
            tc.tile_pool(name="smp", bufs=4) as smp,
            tc.tile_pool(name="psp", bufs=8, space="PSUM") as psp,
        ):
            # ---- persistent tiles ----
            wba = [wbp.tile([128, OTN, NT16, E, 128], BF16, name=f"wb{t}",
                            tag=f"wb{t}") for t in range(CT)]
            wb8 = wbp.tile([128, OTN, NT8, E, CT, 128], BF16, name="wb8",
                           tag="wb8")
            xp = [[xpp.tile([128, H, WP], BF16, name=f"xp{i}_{t}",
                            tag=f"xp{i}_{t}")
                   for t in range(CT)] for i in range(2)]
            x8p = [xpp.tile([128, CT, H, WP], E4, name=f"x8p{i}",
                            tag=f"x8p{i}") for i in range(2)]
            mw = [[mwp.tile([128, OTN, NT16, 128], BF16, name=f"mw{i}_{t}",
                            tag=f"mw{i}_{t}")
                   for t in range(CT)] for i in range(2)]
            mw8 = [mwp.tile([128, OTN, NT8, CT, 128], E4, name=f"mw8{i}",
                            tag=f"mw8{i}") for i in range(2)]
            pp_sb = snp.tile([128, 20], F32, name="pp_sb", tag="pp_sb")
            bn_inv = [snp.tile([128, 1], F32, name=f"bninv{o}", tag=f"bninv{o}")
                      for o in range(OTN)]
            bn_shift = [snp.tile([128, 1], F32, name=f"bnsh{o}", tag=f"bnsh{o}")
                        for o in range(OTN)]
            zeros1 = snp.tile([128, 1], F32, name="zeros1", tag="zeros1")
            pscr = snp.tile([128, H * WP], BF16, name="pscr", tag="pscr")
            warm_w = snp.tile([128, 128], BF16, name="warm_w", tag="warm_w")
            ones_bf = snp.tile([128, 128], BF16, name="ones_bf", tag="ones_bf")
            warm_x = snp.tile([128, NF], BF16, name="warm_x", tag="warm_x")
            tgarb = snp.tile([128, 1], F32, name="tgarb", tag="tgarb")
            gate_sb = snp.tile([128, 4], E4, name="gate_sb",
                               tag="gate_sb")

            # engine-queue preludes: memsets gate the warm-up matmuls; the
            # dummy Sigmoid pulls the ACT function-table load off the
            # routing critical path
            nc.vector.memset(zeros1, 0.0)
            nc.vector.memset(ones_bf, 1.0)
            nc.vector.memset(warm_w, 0.0)
            nc.vector.memset(warm_x, 0.0)
            nc.scalar.activation(out=tgarb, in_=zeros1, func=AF.Sigmoid,
                                 bias=zeros1)

            # the warm-up tile shares the acc tag/shape, so after the
            # preamble its PSUM bank rejoins the conv rotation -- all 8
            # banks serve the steady state (7 + a parked warm bank left
            # sweep boundaries exactly at the pool limit)
            wps = psp.tile([128, NF + 4], F32, name="warm_ps", tag="acc")

            def warm_block(n, rhs):
                for _ in range(n):
                    nc.tensor.matmul(wps[:, 0:rhs.free_size()],
                                     lhsT=warm_w, rhs=rhs,
                                     start=True, stop=True)

            # ---- preamble DMAs, bandwidth-ordered. The 16 DMA engines
            # round-robin packets across every in-flight descriptor, and
            # per-partition packet size sets throughput -- so x8 (one
            # 6.5KB-packet descriptor covering both cin tiles; routing
            # reads it) goes first with only the tiny pp and the center
            # conv tap alongside, and everything else is gated behind a
            # tiny x8-dependent DMA on the sync queue.
            # x8 as partition-halves: two concurrent descriptors, packets
            # stay at the full 6.5KB per-partition row
            nc.sync.dma_start(out=x8p[0][0:64], in_=x8_ap[0, 0:64])
            nc.sync.dma_start(out=x8p[0][64:128], in_=x8_ap[0, 64:128])
            nc.sync.dma_start(out=pp_sb, in_=pp_ap[:, :])
            # center tap ct0 oi0: needed first by the mix, small
            nc.sync.dma_start(out=wba[0][:, 0, 0:1], in_=wt_ap[0, :, 0, 0:1])
            # no gate: the ~8-deep in-flight pipelining of the sync queue
            # (semaphore-lane recycling) self-throttles the stream, and
            # starting the bulk ~7us earlier beats reserving bandwidth
            # for x8 (sample 0 streams x8 for ALL its conv taps, so the
            # PE stream head needs only x8 + the small oi0 tap banks)
            nc.sync.dma_start(out=wba[0][:, 0, 1:NT16],
                              in_=wt_ap[0, :, 0, 1:NT16])
            nc.sync.dma_start(out=wba[1][:, 0], in_=wt_ap[1, :, 0])
            nc.sync.dma_start(out=wb8[:, 0], in_=w8_ap[:, 0])
            nc.sync.dma_start(out=xp[0][0][0:64], in_=x_ap[0, 0:64, :, :])
            nc.sync.dma_start(out=xp[0][0][64:128], in_=x_ap[0, 64:128, :, :])
            nc.sync.dma_start(out=wba[0][:, 1], in_=wt_ap[0, :, 1])
            nc.sync.dma_start(out=wba[1][:, 1], in_=wt_ap[1, :, 1])
            nc.sync.dma_start(out=wb8[:, 1], in_=w8_ap[:, 1])
            nc.sync.dma_start(out=xp[0][1][0:64], in_=x_ap[0, 128:192, :, :])
            nc.sync.dma_start(out=xp[0][1][64:128],
                              in_=x_ap[0, 192:256, :, :])
            nc.sync.dma_start(out=xp[1][0], in_=x_ap[1, 0:128, :, :])
            nc.sync.dma_start(out=xp[1][1], in_=x_ap[1, 128:256, :, :])
            nc.sync.dma_start(out=x8p[1], in_=x8_ap[1])

            # warm-up: A ungated from t~0, B gated on the x8 DMA
            warm_block(WARM_A, warm_x)
            warm_block(WARM_B, x8p[0][:, 0, 0:7, :])

            rwt = [pp_sb[:, 0:4], pp_sb[:, 4:8]]
            rb_bc = pp_sb[:, 8:12]

            def routing0(rt_ps):
                """Sample-0 routing, fed from the fp8 x copy (one big-packet
                DMA; the e4m3 quantization shifts the logits by ~1e-4,
                invisible next to the fp8 conv noise): ACT reduces cin tile
                0 rows 0:40 via Copy+accum, DVE reduces the rest with a
                pairwise-add tree (DVE tensor_add is ~2-4x faster than its
                fp32 reduce), bf16 half-dots on ACT and DVE, partition
                reduce via two accumulating N=4 PE matmuls, sigmoid off
                the PSUM."""
                # per cin tile, BOTH engines share the reduce: ACT gets
                # columns 0:24 (Copy+accum), DVE gets 24:58 via a pairwise
                # tree -- each tile's reduce takes ~1.4us of each engine
                pla = [smp.tile([128, 1], F32, name=f"pla{t}", tag=f"pla{t}")
                       for t in range(CT)]
                pld = [smp.tile([128, 1], F32, name=f"pld{t}", tag=f"pld{t}")
                       for t in range(CT)]
                g1 = smp.tile([128, 28, 34], BF16, name="gapg1", tag="gapg1")
                g2 = smp.tile([128, 14, 34], BF16, name="gapg2", tag="gapg2")
                g3 = smp.tile([128, 7, 34], BF16, name="gapg3", tag="gapg3")
                for t in range(CT):
                    nc.scalar.activation(out=pscr[:, :H * 24],
                                         in_=x8p[0][:, t, :, 0:24],
                                         func=AF.Copy, accum_out=pla[t])
                    nc.vector.tensor_add(g1, x8p[0][:, t, 0:28, 24:58],
                                         x8p[0][:, t, 28:56, 24:58])
                    nc.vector.tensor_add(g2, g1[:, 0:14], g1[:, 14:28])
                    nc.vector.tensor_add(g3, g2[:, 0:7], g2[:, 7:14])
                    nc.vector.reduce_sum(out=pld[t], in_=g3, axis=AX.XY)
                pl0 = smp.tile([128, 1], F32, name="pl0", tag="pl0")
                pl1 = smp.tile([128, 1], F32, name="pl1", tag="pl1")
                nc.vector.tensor_add(pl0, pla[0], pld[0])
                nc.vector.tensor_add(pl1, pla[1], pld[1])
                # half-dots in bf16 (routing logits are tiny; bf16 noise on
                # the partials is ~1e-5 on the logit)
                prod_a = smp.tile([128, E], BF16, name="prod_a", tag="prod_a")
                prod_b = smp.tile([128, E], BF16, name="prod_b", tag="prod_b")
                nc.scalar.activation(out=prod_a, in_=rwt[0], func=AF.Copy,
                                     scale=pl0)
                nc.vector.scalar_tensor_tensor(out=prod_b, in0=rwt[1],
                                               scalar=pl1, in1=rb_bc,
                                               op0=OP.mult, op1=OP.add)
                nc.tensor.matmul(rt_ps[:, NF:NF + E], lhsT=ones_bf,
                                 rhs=prod_a, start=True, stop=False)
                nc.tensor.matmul(rt_ps[:, NF:NF + E], lhsT=ones_bf,
                                 rhs=prod_b, start=False, stop=True)
                # fillers keep the PE busy through the sigmoid+mix window
                for _ in range(FILLERS):
                    nc.tensor.matmul(wps[:, 0:64], lhsT=warm_w,
                                     rhs=warm_x[:, 0:64],
                                     start=True, stop=True)
                rr = smp.tile([128, E], F32, name="rr0", tag="rr")
                nc.scalar.activation(out=rr, in_=rt_ps[:, NF:NF + E],
                                     func=AF.Sigmoid,
                                     scale=1.0 / (H * W), bias=zeros1)
                return rr

            def routing(s):
                """Later samples: ~13us of slack, GAP fully on ACT and the
                partition reduce on gpsimd (its dispatch jitter is harmless
                here and keeps the mid-stream PE FIFO clean)."""
                i = s % 2
                pl = [smp.tile([128, 1], F32, name=f"pl{s}_{t}", tag=f"pl{t}")
                      for t in range(CT)]
                # cin tile 0 on ACT, tile 1 on a DVE pairwise tree: halves
                # the ACT occupancy so the next conv's ACT-side mixes (and
                # with them the PE) don't stall behind the GAP
                nc.scalar.activation(out=pscr[:, :H * WP], in_=xp[i][0],
                                     func=AF.Copy, accum_out=pl[0])
                gr1 = smp.tile([128, 28, WP], BF16, name=f"gr1_{s}",
                               tag="gr1")
                gr2 = smp.tile([128, 14, WP], BF16, name=f"gr2_{s}",
                               tag="gr2")
                gr3 = smp.tile([128, 7, WP], BF16, name=f"gr3_{s}",
                               tag="gr3")
                nc.vector.tensor_add(gr1, xp[i][1][:, 0:28, :],
                                     xp[i][1][:, 28:56, :])
                nc.vector.tensor_add(gr2, gr1[:, 0:14], gr1[:, 14:28])
                nc.vector.tensor_add(gr3, gr2[:, 0:7], gr2[:, 7:14])
                nc.vector.reduce_sum(out=pl[1], in_=gr3, axis=AX.XY)
                prod = smp.tile([128, E], F32, name=f"prod{s}", tag="prod")
                nc.vector.scalar_tensor_tensor(out=prod, in0=rwt[0],
                                               scalar=pl[0], in1=rb_bc,
                                               op0=OP.mult, op1=OP.add)
                nc.vector.scalar_tensor_tensor(out=prod, in0=rwt[1],
                                               scalar=pl[1], in1=prod,
                                               op0=OP.mult, op1=OP.add)
                lg = smp.tile([128, E], F32, name=f"lg{s}", tag="lg")
                nc.gpsimd.partition_all_reduce(
                    lg, prod, channels=128,
                    reduce_op=bass_isa.ReduceOp.add)
                rr = smp.tile([128, E], F32, name=f"rr{s}", tag="rr")
                nc.scalar.activation(out=rr, in_=lg, func=AF.Sigmoid,
                                     scale=1.0 / (H * W), bias=zeros1)
                return rr

            def mix_tree(s, oi, t, a, b, rr):
                """Expert-pair tree mix of bf16 taps a:b into mw."""
                i = s % 2
                c1 = smp.tile([128, b - a, 128], BF16,
                              name=f"mc1_{s}_{oi}_{t}_{a}", tag=f"mc1_{a}")
                c2 = smp.tile([128, b - a, 128], BF16,
                              name=f"mc2_{s}_{oi}_{t}_{a}", tag=f"mc2_{a}")
                nc.vector.tensor_scalar_mul(c1, wba[t][:, oi, a:b, 0, :],
                                            rr[:, 0:1])
                nc.vector.tensor_scalar_mul(c2, wba[t][:, oi, a:b, 2, :],
                                            rr[:, 2:3])
                nc.vector.scalar_tensor_tensor(
                    out=c1, in0=wba[t][:, oi, a:b, 1, :],
                    scalar=rr[:, 1:2], in1=c1, op0=OP.mult, op1=OP.add)
                nc.vector.scalar_tensor_tensor(
                    out=c2, in0=wba[t][:, oi, a:b, 3, :],
                    scalar=rr[:, 3:4], in1=c2, op0=OP.mult, op1=OP.add)
                nc.vector.tensor_add(mw[i][t][:, oi, a:b], c1, c2)

            def mix_dve(s, oi, t, rr):
                """bf16 mix as a 4-deep stt chain (steady-state samples)."""
                i = s % 2
                nc.vector.tensor_scalar_mul(mw[i][t][:, oi],
                                            wba[t][:, oi, :, 0, :],
                                            rr[:, 0:1])
                for e in range(1, E):
                    nc.vector.scalar_tensor_tensor(
                        out=mw[i][t][:, oi],
                        in0=wba[t][:, oi, :, e, :],
                        scalar=rr[:, e:e + 1], in1=mw[i][t][:, oi],
                        op0=OP.mult, op1=OP.add)

            def mix_act(s, oi, t, rr):
                """bf16 mix with expert scaling on ACT (scaled Copy) and
                DVE doing only the adds, so two mix chains overlap."""
                i = s % 2
                ce = [smp.tile([128, NT16, 128], BF16, name=f"ce{s}_{oi}_{e}",
                               tag=f"ce{e}", bufs=2) for e in range(E)]
                for e in range(E):
                    nc.scalar.activation(out=ce[e], in_=wba[t][:, oi, :, e, :],
                                         func=AF.Copy, scale=rr[:, e:e + 1])
                nc.vector.tensor_add(mw[i][t][:, oi], ce[0], ce[1])
                nc.vector.tensor_add(mw[i][t][:, oi], mw[i][t][:, oi], ce[2])
                nc.vector.tensor_add(mw[i][t][:, oi], mw[i][t][:, oi], ce[3])

            def mix8(s, oi, rr):
                """fp8 corner-tap mix: expert-pair tree over the whole
                [128, NT8, CT, 128] slab, single e4m3 rounding on the
                final add."""
                i = s % 2
                c1 = smp.tile([128, NT8, CT, 128], BF16, name=f"m81_{s}_{oi}",
                              tag="m81", bufs=2)
                c2 = smp.tile([128, NT8, CT, 128], BF16, name=f"m82_{s}_{oi}",
                              tag="m82", bufs=2)
                nc.vector.tensor_scalar_mul(c1, wb8[:, oi, :, 0], rr[:, 0:1])
                nc.vector.tensor_scalar_mul(c2, wb8[:, oi, :, 2], rr[:, 2:3])
                nc.vector.scalar_tensor_tensor(
                    out=c1, in0=wb8[:, oi, :, 1], scalar=rr[:, 1:2], in1=c1,
                    op0=OP.mult, op1=OP.add)
                nc.vector.scalar_tensor_tensor(
                    out=c2, in0=wb8[:, oi, :, 3], scalar=rr[:, 3:4], in1=c2,
                    op0=OP.mult, op1=OP.add)
                nc.vector.tensor_add(mw8[i][:, oi], c1, c2)

            def bn_fold():
                # inv = (gamma/64) / sqrt(var+eps); shift = beta - 64*mean
                # * inv (the host pre-folds the 1/WSCALE). rsqrt via linear
                # seed + 2 Newton steps (var bounded in [0.5, 1.5]) -- pure
                # DVE, no ACT Sqrt table thrash. Gated on the ct1 mix so
                # the scheduler can't wedge it into the routing window.
                gate = smp.tile([128, 1], F32, name="bngate", tag="bngate")
                nc.vector.tensor_scalar_mul(gate,
                                            mw[0][1][:, 0, NT16 - 1, 0:1],
                                            0.0)
                for o in range(OTN):
                    p = 12 + 4 * o
                    va = smp.tile([128, 1], F32, name=f"va{o}", tag=f"va{o}")
                    nc.vector.tensor_scalar_add(va, pp_sb[:, p + 3:p + 4],
                                                BN_EPS)
                    ve = smp.tile([128, 1], F32, name=f"ve{o}", tag=f"ve{o}")
                    nc.vector.tensor_add(ve, va, gate)
                    r = bn_inv[o]
                    nc.vector.tensor_scalar(r, ve, -0.5977, 1.6561,
                                            op0=OP.mult, op1=OP.add)
                    t = smp.tile([128, 1], F32, name=f"nt{o}", tag=f"nt{o}")
                    for _ in range(2):
                        nc.vector.tensor_mul(t, r, r)
                        nc.vector.tensor_mul(t, t, ve)
                        nc.vector.tensor_scalar(t, t, -0.5, 1.5,
                                                op0=OP.mult, op1=OP.add)
                        nc.vector.tensor_mul(r, r, t)
                    nc.vector.tensor_mul(bn_inv[o], r, pp_sb[:, p:p + 1])
                    mi = smp.tile([128, 1], F32, name=f"mi{o}", tag=f"mi{o}")
                    nc.vector.tensor_mul(mi, pp_sb[:, p + 2:p + 3], bn_inv[o])
                    nc.vector.tensor_sub(bn_shift[o], pp_sb[:, p + 1:p + 2],
                                         mi)

            def mm16(s, oi, t, k, r0, nr, acc, start, stop, rhs_x8=False):
                i = s % 2
                ki, kj = TAPS16[k]
                h_lo = max(r0, 1 - ki)
                h_hi = min(r0 + nr - 1, H - ki)
                if rhs_x8:
                    # sample-0 sweep only: the cin-tile-1 taps stream the
                    # fp8 x copy (already resident) so the first conv
                    # matmuls don't wait for the second bf16 x transfer;
                    # costs ~1e-4 extra rel-err overall
                    rhs = x8p[i][:, t, h_lo + ki - 1:h_hi + ki,
                                XO - 1 + kj:XO - 1 + kj + W]
                else:
                    rhs = xp[i][t][:, h_lo + ki - 1:h_hi + ki,
                                   XO - 1 + kj:XO - 1 + kj + W]
                nc.tensor.matmul(
                    acc[:, (h_lo - r0) * W:(h_hi - r0 + 1) * W],
                    lhsT=mw[i][t][:, oi, k, :],
                    rhs=rhs, start=start, stop=stop)

            def mm8(s, oi, j, r0, nr, acc, start, stop):
                i = s % 2
                ki, kj = TAPS8[j]
                h_lo = max(r0, 1 - ki)
                h_hi = min(r0 + nr - 1, H - ki)
                nc.tensor.matmul(
                    acc[:, (h_lo - r0) * W:(h_hi - r0 + 1) * W],
                    lhsT=mw8[i][:, oi, j],
                    rhs=x8p[i][:, :, h_lo + ki - 1:h_hi + ki,
                               XO - 1 + kj:XO - 1 + kj + W],
                    start=start, stop=stop, perf_mode=DR)

            def evac(s, oi, r0, nr, acc):
                """BN+ReLU on ACT (PSUM read), residual add on DVE (bf16
                2x), bf16 store."""
                i = s % 2
                ob = otp.tile([128, NF], BF16, name=f"ob{s}_{oi}_{r0}",
                              tag="ob")
                obs = ob[:, :nr * W]
                nc.scalar.activation(out=obs, in_=acc[:, :nr * W],
                                     func=AF.Relu,
                                     bias=bn_shift[oi], scale=bn_inv[oi])
                ob3 = obs.rearrange("p (a b) -> p a b", a=nr)
                nc.vector.tensor_add(ob3, ob3,
                                     xp[i][oi][:, r0:r0 + nr, XO:XO + W])
                nc.sync.dma_start(out=y_ap[s, oi * 128:oi * 128 + 128,
                                           r0:r0 + nr, :],
                                  in_=ob3)

            def chunk_taps(r0, nr):
                """Tap sequences with empty (fully row-clipped) taps
                dropped; bf16 first (center leads: full coverage for the
                start=True has_written clear), fp8 last."""
                b16, f8 = [], []
                for t in range(CT):
                    for kt in range(NT16):
                        ki, _ = TAPS16[kt]
                        if min(r0 + nr - 1, H - ki) >= max(r0, 1 - ki):
                            b16.append((t, kt))
                for j in range(NT8):
                    ki, _ = TAPS8[j]
                    if min(r0 + nr - 1, H - ki) >= max(r0, 1 - ki):
                        f8.append(j)
                return b16, f8

            def conv(s, oi):
                """One output channel tile, row chunks processed in pairs:
                [A bf16][B bf16][A fp8 + evac][B fp8 + evac]. Pairing
                halves the bf16->DoubleRow weight-path switches (the first
                DR matmul after a bf16 run pays ~170ns). The kernel's very
                last chunk is split so its evacuation tail is shorter."""
                chunks = [(rc * RH, RH) for rc in range(RC)]
                if s == BPC - 1 and oi == 1:
                    # final sweep: tiny bottom chunks go FIRST (their evacs
                    # hide under the remaining matmuls) and the kernel ends
                    # on a 4-row chunk, so the tail is one short
                    # ACT+DVE+DMA chain
                    chunks = ([(48, 5), (53, 2), (55, 1)] + chunks[:-1]
                              )[:-1] + [(40, 4), (44, 4)]
                groups, i0 = [], 0
                while i0 < len(chunks):
                    n = 4 if not groups else 3
                    groups.append(chunks[i0:i0 + n])
                    i0 += n
                for pair in groups:
                    accs = [psp.tile([128, NF + 4], F32,
                                     name=f"acc{s}_{oi}_{r0}", tag="acc")
                            for r0, nr in pair]
                    for (r0, nr), acc in zip(pair, accs):
                        for k, (t, kt) in enumerate(chunk_taps(r0, nr)[0]):
                            # sample 0 streams the fp8 x copy for its
                            # cin-tile-1 bf16 taps, so no PE wait in the
                            # first ~35us depends on the second bf16 x
                            # transfer (whose coalesced wait would stall
                            # the stream head)
                            mm16(s, oi, t, kt, r0, nr, acc, start=(k == 0),
                                 stop=False, rhs_x8=(s == 0))
                    for (r0, nr), acc in zip(pair, accs):
                        f8 = chunk_taps(r0, nr)[1]
                        for n, j in enumerate(f8):
                            mm8(s, oi, j, r0, nr, acc, start=False,
                                stop=(n == len(f8) - 1))
                        evac(s, oi, r0, nr, acc)

            def conv_sweep(s, oi):
                """First conv: all bf16 taps for all chunks (ct0 in per-tap
                blocks tracking the fine-grained mix, then ct1), then the
                fp8 taps chunk-by-chunk with interleaved evacuation. Keeps
                the PE fed while the later mixes and the fp8 inputs land.
                Uses 7 PSUM banks (+1 warm-up) = all 8."""
                accs = acc0
                for t in range(CT):
                    for kt in range(NT16):
                        for rc in range(RC):
                            mm16(s, oi, t, kt, rc * RH, RH, accs[rc],
                                 start=(t == 0 and kt == 0), stop=False,
                                 rhs_x8=True)
                for rc in range(RC):
                    for j in range(NT8):
                        mm8(s, oi, j, rc * RH, RH, accs[rc], start=False,
                            stop=(j == NT8 - 1))
                    evac(s, oi, rc * RH, RH, accs[rc])

            # ---- program ----
            acc0 = [psp.tile([128, NF + 4], F32, name=f"acc0_0_{rc}",
                             tag="acc") for rc in range(RC)]
            rr0 = routing0(rt_ps=acc0[6])
            # sample 0, oi 0: fine-grained per-tap mixing on DVE for ct0;
            # ct1 via ACT scaled copies (ACT is idle after the sigmoid)
            mix_tree(0, 0, 0, 0, 1, rr0)
            mix_tree(0, 0, 0, 1, 2, rr0)
            mix_tree(0, 0, 0, 2, NT16, rr0)
            mix_act(0, 0, 1, rr0)
            mix8(0, 0, rr0)
            bn_fold()
            rrs = {0: rr0}

            for s in range(BPC):
                if s >= 1 and s + 1 < BPC:
                    nc.sync.dma_start(out=xp[(s + 1) % 2][0],
                                      in_=x_ap[s + 1, 0:128, :, :])
                    nc.sync.dma_start(out=xp[(s + 1) % 2][1],
                                      in_=x_ap[s + 1, 128:256, :, :])
                    nc.sync.dma_start(out=x8p[(s + 1) % 2],
                                      in_=x8_ap[s + 1])
                if s == 0:
                    conv_sweep(0, 0)
                    mix_dve(0, 1, 0, rr0)
                    mix_act(0, 1, 1, rr0)
                    mix8(0, 1, rr0)
                else:
                    conv(s, 0)
                if s + 1 < BPC:
                    rr = routing(s + 1)
                    rrs[s + 1] = rr
                    mix_dve(s + 1, 0, 0, rr)
                    mix_act(s + 1, 0, 1, rr)
                    mix8(s + 1, 0, rr)
                conv(s, 1)
                if s + 1 < BPC:
                    rr = rrs[s + 1]
                    mix_dve(s + 1, 1, 0, rr)
                    mix_act(s + 1, 1, 1, rr)
                    mix8(s + 1, 1, rr)

    nc.compile()
    return nc


_CACHE = {}
_LOCK = threading.Lock()


def prepare_in_maps(inputs):
    """Host-side layout prep (sharding + transposes + dtype casts only)."""
    x = np.asarray(inputs["x"], dtype=np.float32)
    route_w = np.asarray(inputs["route_w"], dtype=np.float32)
    route_b = np.asarray(inputs["route_b"], dtype=np.float32)
    expert_w = np.asarray(inputs["expert_w"], dtype=np.float32)
    bn_gamma = np.asarray(inputs["bn_gamma"], dtype=np.float32)
    bn_beta = np.asarray(inputs["bn_beta"], dtype=np.float32)
    bn_mean = np.asarray(inputs["bn_mean"], dtype=np.float32)
    bn_var = np.asarray(inputs["bn_var"], dtype=np.float32)

    ws = expert_w * WSCALE
    # bf16 taps: [E, COUT, CIN, K, K] -> [CT, 128cin, OTN, tap, E, 128cout]
    a = ws.reshape(E, OTN, 128, CIN, KS, KS)
    b = a.transpose(3, 1, 4, 5, 0, 2).reshape(CIN, OTN, KS * KS, E, 128)
    kki16 = [ki * KS + kj for ki, kj in TAPS16]
    wt = np.ascontiguousarray(
        b[:, :, kki16].reshape(CT, 128, OTN, NT16, E, 128)
    ).astype(ml_dtypes.bfloat16)
    # fp8 corner taps, DoubleRow pairing over cin tiles:
    # [128cin, OTN, tap, E, CT, 128cout]
    kki8 = [ki * KS + kj for ki, kj in TAPS8]
    c = b[:, :, kki8].reshape(CT, 128, OTN, NT8, E, 128)
    w8 = np.ascontiguousarray(c.transpose(1, 2, 3, 4, 0, 5)).astype(
        ml_dtypes.bfloat16)

    # packed params [128, 20]: rwt t0 | rwt t1 | rb (replicated) | bn o0 |
    # bn o1 (gamma/WSCALE, beta, WSCALE*mean, var columns -- the BN scale
    # absorbs the weight pre-scale)
    rwt = route_w.T  # [CIN, E]
    bnp = np.stack([bn_gamma / WSCALE, bn_beta, bn_mean * WSCALE, bn_var],
                   axis=1)
    rb_fold = route_b * (H * W) / 128.0
    pp = np.concatenate([rwt[0:128], rwt[128:256],
                         np.tile(rb_fold[None, :], (128, 1)),
                         bnp[0:128], bnp[128:256]], axis=1)
    pp = np.ascontiguousarray(pp.astype(np.float32))

    # width-pad on host: border halos arrive pre-zeroed
    xpad = np.zeros((B, CIN, H, WP), dtype=np.float32)
    xpad[:, :, :, XO:XO + W] = x
    x16 = xpad.astype(ml_dtypes.bfloat16)
    # fp8 copy, cin tiles paired along a free axis: [B, 128, CT, H, WP]
    x8 = np.ascontiguousarray(
        xpad.reshape(B, CT, 128, H, WP).transpose(0, 2, 1, 3, 4)
    ).astype(ml_dtypes.float8_e4m3)

    return [
        {"x": np.ascontiguousarray(x16[c * BPC:(c + 1) * BPC]),
         "x8": np.ascontiguousarray(x8[c * BPC:(c + 1) * BPC]),
         "wt": wt, "w8": w8, "pp": pp}
        for c in range(N_CORES)
    ]


def _get_nc():
    with _LOCK:
        if "nc" not in _CACHE:
            _CACHE["nc"] = build_bass()
        return _CACHE["nc"]


def kernel(**inputs):
    in_maps = prepare_in_maps(inputs)
    nc = _get_nc()
    res = run_bass_kernel_spmd(nc, in_maps, core_ids=list(range(N_CORES)))
    return np.concatenate([np.asarray(r["y"], dtype=np.float32)
                           for r in res.results], axis=0)
